# revision 1
# baseline (speedup 1.0000x reference)
"""DGCNN Trainium kernel: per-core one batch (B=4 over 4+4 cores).

Phases per core/batch (sequential, pools scoped per phase):
  S: pd scores via PE (K=4 fp32) -> PSUM (4-bank halves); segmax16 (DVE);
     top-24 segments (max8 rounds); indirect-gather candidate segments;
     DVE rescore; top-20 + global neighbor ids; neighbor gather.
  F: n-major feature math (19 ch) in half batches + PE transpose to
     channel-major F_cm [(kk4,c32) x n].
  M: recompute MLP passes A-D, GN stats via bn_stats + PE-onehot
     reductions, normalize+relu on ACT, final running max over k.
"""
import numpy as np
from contextlib import ExitStack

import concourse.bass as bass
import concourse.tile as tile
from concourse import mybir

dt = mybir.dt
F32, U32, I32 = dt.float32, dt.uint32, dt.int32
AF = mybir.ActivationFunctionType
OP = mybir.AluOpType
AX = mybir.AxisListType

N = 4096
NBLK_FULL = 32
K = 20
SEGW = 16
NSEG = 256
NCS = 24
C1, C2, C3 = 64, 64, 96
GN_EPS = 1e-5
NKK_G = 5
NCAND = NCS * SEGW


def host_prep(data_b, W1, g1, b1, W2, g2, b2, W3, g3, b3):
    """Per-batch host tables (layout prep only). data_b: (6, N) f32."""
    x = data_b[:3].astype(np.float32)
    nrm = data_b[3:6].astype(np.float32)
    xx = ((x[0] * x[0] + x[1] * x[1]) + x[2] * x[2]).astype(np.float32)
    qtab = np.stack([2 * x[0], 2 * x[1], 2 * x[2],
                     np.ones(N, np.float32)]).astype(np.float32)
    rtab = np.stack([x[0], x[1], x[2], -xx]).astype(np.float32)
    seg = np.zeros((NSEG, SEGW, 4), np.float32)
    seg[:, :, 0] = x[0].reshape(NSEG, SEGW)
    seg[:, :, 1] = x[1].reshape(NSEG, SEGW)
    seg[:, :, 2] = x[2].reshape(NSEG, SEGW)
    seg[:, :, 3] = xx.reshape(NSEG, SEGW)
    seg_tab = seg.reshape(NSEG, 64)
    pt = np.zeros((N, 8), np.float32)
    pt[:, 0:3] = x.T
    pt[:, 3:6] = nrm.T
    pt[:, 6] = xx
    ctr = x.T.reshape(NBLK_FULL, 128, 3).transpose(1, 0, 2)
    ctr2 = np.ascontiguousarray(2.0 * ctr).astype(np.float32)
    cnrm = np.ascontiguousarray(nrm.T.reshape(NBLK_FULL, 128, 3).transpose(1, 0, 2))
    iota24 = np.broadcast_to(np.arange(NCS, dtype=np.float32), (128, NCS)).copy()
    idn = np.eye(128, dtype=np.float32)
    W1p = np.zeros((32, C1), np.float32)
    W1p[:19, :] = W1.T
    w1a = np.zeros((128, 128), np.float32)
    w1b = np.zeros((128, 128), np.float32)
    for kk in range(2):
        w1a[kk * 32:(kk + 1) * 32, kk * 64:(kk + 1) * 64] = W1p
        w1b[(kk + 2) * 32:(kk + 3) * 32, kk * 64:(kk + 1) * 64] = W1p
    w2bd = np.zeros((128, 128), np.float32)
    w2bd[:64, :64] = W2.T
    w2bd[64:, 64:] = W2.T
    w3t = np.ascontiguousarray(np.vstack([W3.T, W3.T]))  # [128, 96]
    m1_12 = np.zeros((128, 16), np.float32)
    for p in range(128):
        m1_12[p, (p % 64) // 4] = 1.0
    e_12 = np.zeros((16, 128), np.float32)
    for p in range(128):
        e_12[(p % 64) // 4, p] = 1.0
    m1_3 = np.zeros((96, 16), np.float32)
    for p in range(96):
        m1_3[p, p // 6] = 1.0
    e_3 = np.zeros((16, 96), np.float32)
    for p in range(96):
        e_3[p // 6, p] = 1.0
    g1rep = np.tile(g1, 2).reshape(128, 1).astype(np.float32)
    b1rep = np.tile(b1, 2).reshape(128, 1).astype(np.float32)
    g2rep = np.tile(g2, 2).reshape(128, 1).astype(np.float32)
    b2rep = np.tile(b2, 2).reshape(128, 1).astype(np.float32)
    g3rep = g3.reshape(96, 1).astype(np.float32)
    b3rep = b3.reshape(96, 1).astype(np.float32)
    return {
        "qtab": qtab, "rtab": rtab, "seg_tab": seg_tab, "pt_tab": pt,
        "ctr2": ctr2, "cnrm": cnrm, "iota24": iota24, "idn": idn,
        "w1a": w1a, "w1b": w1b, "w2bd": w2bd, "w3t": w3t,
        "m1_12": m1_12, "e_12": e_12, "m1_3": m1_3, "e_3": e_3,
        "g1rep": g1rep, "b1rep": b1rep, "g2rep": g2rep, "b2rep": b2rep,
        "g3rep": g3rep, "b3rep": b3rep,
    }


INPUT_SHAPES = {
    "qtab": (4, N), "rtab": (4, N), "seg_tab": (NSEG, 64), "pt_tab": (N, 8),
    "ctr2": (128, NBLK_FULL, 3), "cnrm": (128, NBLK_FULL, 3),
    "iota24": (128, NCS), "idn": (128, 128),
    "w1a": (128, 128), "w1b": (128, 128), "w2bd": (128, 128), "w3t": (128, 96),
    "m1_12": (128, 16), "e_12": (16, 128), "m1_3": (96, 16), "e_3": (16, 96),
    "g1rep": (128, 1), "b1rep": (128, 1), "g2rep": (128, 1), "b2rep": (128, 1),
    "g3rep": (96, 1), "b3rep": (96, 1),
}


def declare_inputs(nc):
    return {k: nc.dram_tensor(k, list(sh), F32, kind="ExternalInput").ap()
            for k, sh in INPUT_SHAPES.items()}


def build(nc, tc, ctx, din, out_ap, nblk=NBLK_FULL, dbg=None, phases="SFM"):
    NI = nblk * 128
    NIK = NI * K
    dbg = dbg or {}

    consts = ctx.enter_context(tc.tile_pool(name="consts", bufs=1))
    gathp = ctx.enter_context(tc.tile_pool(name="gath", bufs=1))
    fcmp = ctx.enter_context(tc.tile_pool(name="fcm", bufs=1))

    ld = {}
    for name in ["qtab", "rtab", "ctr2", "cnrm", "iota24", "idn", "w1a", "w1b",
                 "w2bd", "w3t", "m1_12", "e_12", "m1_3", "e_3",
                 "g1rep", "b1rep", "g2rep", "b2rep", "g3rep", "b3rep"]:
        t = consts.tile(list(INPUT_SHAPES[name]), F32, tag=name)
        nc.gpsimd.dma_start(t[:], din[name][:])
        ld[name] = t

    gath = gathp.tile([128, nblk, K, 8], F32)
    fcm = fcmp.tile([128, NKK_G, NI], F32)

    # ================= Phase S =================
    with ExitStack() as sctx:
        selp = sctx.enter_context(tc.tile_pool(name="sel", bufs=2))
        pspd = sctx.enter_context(tc.tile_pool(name="pspd", bufs=1, space="PSUM"))

        import os as _os
        SLVL = int(_os.environ.get("SLVL", "11"))
        for blk in range(nblk):
            segmax = selp.tile([128, NSEG], F32, tag="segmax")
            for h2 in range(2):
                pd = pspd.tile([128, NSEG // 2, SEGW], F32, tag="pd")
                for ch in range(4):
                    nc.tensor.matmul(
                        pd[:].rearrange("p s w -> p (s w)")[:, ch * 512:(ch + 1) * 512],
                        ld["qtab"][:, blk * 128:(blk + 1) * 128],
                        ld["rtab"][:, h2 * 2048 + ch * 512:h2 * 2048 + (ch + 1) * 512],
                        start=True, stop=True)
                nc.vector.tensor_reduce(segmax[:, h2 * 128:(h2 + 1) * 128], pd[:], AX.X, OP.max)

            if SLVL < 2:
                if blk == 0:
                    nc.gpsimd.dma_start(out_ap[:, :NSEG], segmax[:96, :])
                continue
            segv = selp.tile([128, NCS], F32, tag="segv")
            segi = selp.tile([128, NCS], U32, tag="segi")
            for r in range(3):
                nc.vector.max(segv[:, r * 8:(r + 1) * 8], segmax[:])
                nc.vector.max_index(segi[:, r * 8:(r + 1) * 8], segv[:, r * 8:(r + 1) * 8], segmax[:])
                if r < 2:
                    nc.vector.match_replace(segmax[:], segv[:, r * 8:(r + 1) * 8], segmax[:], -1e30)

            segi32 = selp.tile([128, NCS], I32, tag="segi32")
            nc.vector.tensor_copy(segi32[:], segi[:])
            segf = selp.tile([128, NCS], F32, tag="segf")
            nc.vector.tensor_copy(segf[:], segi[:])

            if SLVL < 3:
                if blk == 0:
                    nc.gpsimd.dma_start(out_ap[:, :NCS], segf[:96, :])
                continue
            cand = selp.tile([128, NCS, 64], F32, tag="cand")
            for c in range(NCS):
                nc.gpsimd.indirect_dma_start(
                    out=cand[:, c, :], out_offset=None, in_=din["seg_tab"][:],
                    in_offset=bass.IndirectOffsetOnAxis(ap=segi32[:, c:c + 1], axis=0))

            if SLVL < 4:
                if blk == 0:
                    nc.gpsimd.dma_start(out_ap[:, :64], cand[:96, 0, :])
                continue
            cxyz = cand[:].rearrange("p c (m d) -> p (c m) d", d=4)
            sc = selp.tile([128, NCAND], F32, tag="scores")
            q0 = ld["ctr2"][:, blk, 0:1]
            q1 = ld["ctr2"][:, blk, 1:2]
            q2 = ld["ctr2"][:, blk, 2:3]
            nc.vector.tensor_scalar(sc[:], cxyz[:, :, 0], q0, None, OP.mult)
            nc.vector.scalar_tensor_tensor(sc[:], cxyz[:, :, 1], q1, sc[:], OP.mult, OP.add)
            nc.vector.scalar_tensor_tensor(sc[:], cxyz[:, :, 2], q2, sc[:], OP.mult, OP.add)
            nc.vector.scalar_tensor_tensor(sc[:], cxyz[:, :, 3], -1.0, sc[:], OP.mult, OP.add)

            if SLVL < 5:
                if blk == 0:
                    nc.gpsimd.dma_start(out_ap[:, :NCAND], sc[:96, :])
                continue
            canv = selp.tile([128, NCS], F32, tag="canv")
            cani = selp.tile([128, NCS], U32, tag="cani")
            for r in range(3):
                nc.vector.max(canv[:, r * 8:(r + 1) * 8], sc[:])
                nc.vector.max_index(cani[:, r * 8:(r + 1) * 8], canv[:, r * 8:(r + 1) * 8], sc[:])
                if r < 2:
                    nc.vector.match_replace(sc[:], canv[:, r * 8:(r + 1) * 8], sc[:], -1e30)

            if SLVL < 6:
                if blk == 0:
                    nc.gpsimd.dma_start(out_ap[:, :NCS], canv[:96, :])
                continue
            slot = selp.tile([128, NCS], U32, tag="slot")
            memb = selp.tile([128, NCS], U32, tag="memb")
            nc.vector.tensor_scalar(slot[:], cani[:], 4, None, OP.logical_shift_right)
            nc.vector.tensor_scalar(memb[:], cani[:], 15, None, OP.bitwise_and)
            slotf = selp.tile([128, NCS], F32, tag="slotf")
            membf = selp.tile([128, NCS], F32, tag="membf")
            nc.vector.tensor_copy(slotf[:], slot[:])
            nc.vector.tensor_copy(membf[:], memb[:])
            if SLVL < 7:
                if blk == 0:
                    nc.gpsimd.dma_start(out_ap[:, :NCS], membf[:96, :])
                continue
            eq = selp.tile([128, K, NCS], F32, tag="eq")
            nc.vector.tensor_tensor(
                eq[:], slotf[:, :K].rearrange("p r -> p r ()").broadcast_to([128, K, NCS]),
                ld["iota24"][:].rearrange("p c -> p () c").broadcast_to([128, K, NCS]),
                OP.is_equal)
            nc.vector.tensor_tensor(
                eq[:], eq[:],
                segf[:].rearrange("p c -> p () c").broadcast_to([128, K, NCS]), OP.mult)
            segsel = selp.tile([128, K], F32, tag="segsel")
            nc.vector.tensor_reduce(segsel[:], eq[:], AX.X, OP.add)
            if SLVL < 8:
                if blk == 0:
                    nc.gpsimd.dma_start(out_ap[:, :K], segsel[:96, :])
                continue
            nbrf = selp.tile([128, K], F32, tag="nbrf")
            nc.vector.scalar_tensor_tensor(nbrf[:], segsel[:], 16.0, membf[:, :K], OP.mult, OP.add)
            nbri = selp.tile([128, K], I32, tag="nbri")
            nc.vector.tensor_copy(nbri[:], nbrf[:])

            if SLVL < 9:
                if blk == 0:
                    nc.gpsimd.dma_start(out_ap[:, :K], nbrf[:96, :])
                continue
            if SLVL < 10:
                if blk == 0:
                    nc.gpsimd.dma_start(dbg["nbrf"][:], nbrf[:])
                continue
            if "nbrf" in dbg and blk == 0:
                nc.gpsimd.dma_start(dbg["nbrf"][:], nbrf[:])

            for kk in range(K):
                nc.gpsimd.indirect_dma_start(
                    out=gath[:, blk, kk, :], out_offset=None, in_=din["pt_tab"][:],
                    in_offset=bass.IndirectOffsetOnAxis(ap=nbri[:, kk:kk + 1], axis=0))

    if "gath" in dbg:
        nc.gpsimd.dma_start(dbg["gath"][:], gath[:])

    if "F" not in phases:
        import os as _os2
        if int(_os2.environ.get("SLVL", "11")) >= 11:
            nc.gpsimd.dma_start(out_ap[:, :40], gath[:96, 0, :5, :])
        return
    # ================= Phase F + transposes =================
    half_nb = min(nblk, 8)
    with ExitStack() as fctx:
        featp = fctx.enter_context(tc.tile_pool(name="feat", bufs=1))
        pstp = fctx.enter_context(tc.tile_pool(name="pstp", bufs=2, space="PSUM"))

        for half in range((nblk + half_nb - 1) // half_nb):
            b0 = half * half_nb
            b1 = min(nblk, b0 + half_nb)
            nb_h = b1 - b0
            hw = nb_h * K
            F = featp.tile([128, half_nb, K, 32], F32, tag="F")
            nc.vector.memset(F[:, :, :, 19:32], 0.0)
            t = featp.tile([128, 8, half_nb * K], F32, tag="scratch")

            g3v = gath[:, b0:b1]                       # [p, b, k, 8]
            gx = g3v[:, :, :, 0]; gy = g3v[:, :, :, 1]; gz = g3v[:, :, :, 2]
            nx = g3v[:, :, :, 3]; ny = g3v[:, :, :, 4]; nz = g3v[:, :, :, 5]

            def ctrb(d):
                return ld["ctr2"][:, b0:b1, d:d + 1].broadcast_to([128, nb_h, K])

            def cnb(d):
                return ld["cnrm"][:, b0:b1, d:d + 1].broadcast_to([128, nb_h, K])

            def ch(i):
                return F[:, :nb_h, :, i]

            def tv(i):
                return t[:, i, :hw].rearrange("p (b k) -> p b k", k=K)

            # ch0-2 gxyz ; ch3-5 xc = 0.5*ctr2 ; ch6-8 lxyz
            for d, g_ in enumerate([gx, gy, gz]):
                nc.vector.tensor_copy(ch(d), g_)
                nc.vector.tensor_scalar(ch(3 + d), ctrb(d), 0.5, None, OP.mult)
                nc.vector.tensor_tensor(ch(6 + d), g_, ch(3 + d), OP.subtract)

            def emit_angle(out_ap, v1, v2, r_ap):
                cx_, cy_, cz_, dot_, y2_ = tv(1), tv(2), tv(3), tv(4), tv(5)
                a_, b_ = tv(6), tv(7)
                nc.vector.tensor_tensor(a_, v1[1], v2[2], OP.mult)
                nc.vector.tensor_tensor(b_, v1[2], v2[1], OP.mult)
                nc.vector.tensor_tensor(cx_, a_, b_, OP.subtract)
                nc.vector.tensor_tensor(a_, v1[2], v2[0], OP.mult)
                nc.vector.tensor_tensor(b_, v1[0], v2[2], OP.mult)
                nc.vector.tensor_tensor(cy_, a_, b_, OP.subtract)
                nc.vector.tensor_tensor(a_, v1[0], v2[1], OP.mult)
                nc.vector.tensor_tensor(b_, v1[1], v2[0], OP.mult)
                nc.vector.tensor_tensor(cz_, a_, b_, OP.subtract)
                nc.vector.tensor_tensor(y2_, cx_, cx_, OP.mult)
                nc.vector.tensor_tensor(a_, cy_, cy_, OP.mult)
                nc.vector.tensor_tensor(y2_, y2_, a_, OP.add)
                nc.vector.tensor_tensor(a_, cz_, cz_, OP.mult)
                nc.vector.tensor_tensor(y2_, y2_, a_, OP.add)
                nc.scalar.activation(y2_, y2_, AF.Sqrt)
                nc.vector.tensor_tensor(dot_, v1[0], v2[0], OP.mult)
                nc.vector.tensor_tensor(a_, v1[1], v2[1], OP.mult)
                nc.vector.tensor_tensor(dot_, dot_, a_, OP.add)
                nc.vector.tensor_tensor(a_, v1[2], v2[2], OP.mult)
                nc.vector.tensor_tensor(dot_, dot_, a_, OP.add)
                nc.vector.tensor_tensor(dot_, dot_, r_ap, OP.add)
                nc.vector.tensor_scalar(dot_, dot_, 1e-30, None, OP.max)
                nc.vector.reciprocal(dot_, dot_)
                nc.vector.tensor_tensor(a_, y2_, dot_, OP.mult)
                # q = a_ >= 0 ; atan(q) via two-range identity (ACT domain |x|<=pi/2)
                nc.vector.reciprocal(b_, a_)
                nc.vector.tensor_tensor(cx_, a_, b_, OP.min)
                nc.scalar.activation(out_ap, cx_, AF.Arctan)
                nc.vector.tensor_scalar(b_, a_, 1.0, None, OP.is_gt)
                nc.vector.tensor_scalar(cx_, out_ap, -2.0, 1.5707963267948966, OP.mult, OP.add)
                nc.vector.tensor_tensor(cx_, cx_, b_, OP.mult)
                nc.vector.tensor_tensor(out_ap, out_ap, cx_, OP.add)
                nc.vector.tensor_scalar(out_ap, out_ap, 2.0, None, OP.mult)

            # d_norm (ch12)
            d2 = tv(0)
            a0 = tv(6)
            nc.vector.tensor_tensor(d2, ch(6), ch(6), OP.mult)
            nc.vector.tensor_tensor(a0, ch(7), ch(7), OP.mult)
            nc.vector.tensor_tensor(d2, d2, a0, OP.add)
            nc.vector.tensor_tensor(a0, ch(8), ch(8), OP.mult)
            nc.vector.tensor_tensor(d2, d2, a0, OP.add)
            nc.scalar.activation(ch(12), d2, AF.Sqrt)

            emit_angle(ch(9), (cnb(0), cnb(1), cnb(2)), (ch(6), ch(7), ch(8)), ch(12))
            emit_angle(ch(10), (nx, ny, nz), (ch(6), ch(7), ch(8)), ch(12))
            ones_ = featp.tile([128, 1], F32, tag="ones")
            nc.vector.memset(ones_[:], 1.0)
            emit_angle(ch(11), (cnb(0), cnb(1), cnb(2)), (nx, ny, nz),
                       ones_[:].rearrange("p o -> p o ()").broadcast_to([128, nb_h, K]))

            cm = featp.tile([128, 3, half_nb], F32, tag="cm")
            nr = featp.tile([128, 3, half_nb], F32, tag="nr")
            l_nr = featp.tile([128, half_nb], F32, tag="lnr")
            l2 = featp.tile([128, half_nb], F32, tag="l2")
            for d, g_ in enumerate([gx, gy, gz]):
                nc.vector.tensor_reduce(cm[:, d, :nb_h], g_, AX.X, OP.add)
                nc.vector.tensor_scalar(cm[:, d, :nb_h], cm[:, d, :nb_h], 1.0 / K, None, OP.mult)
                nc.vector.scalar_tensor_tensor(nr[:, d, :nb_h], ld["ctr2"][:, b0:b1, d], -0.5,
                                               cm[:, d, :nb_h], OP.mult, OP.add)
            nc.vector.tensor_tensor(l2[:, :nb_h], nr[:, 0, :nb_h], nr[:, 0, :nb_h], OP.mult)
            nc.vector.tensor_tensor(l_nr[:, :nb_h], nr[:, 1, :nb_h], nr[:, 1, :nb_h], OP.mult)
            nc.vector.tensor_tensor(l2[:, :nb_h], l2[:, :nb_h], l_nr[:, :nb_h], OP.add)
            nc.vector.tensor_tensor(l_nr[:, :nb_h], nr[:, 2, :nb_h], nr[:, 2, :nb_h], OP.mult)
            nc.vector.tensor_tensor(l2[:, :nb_h], l2[:, :nb_h], l_nr[:, :nb_h], OP.add)
            nc.scalar.activation(l_nr[:, :nb_h], l2[:, :nb_h], AF.Sqrt)

            def nrb(d):
                return nr[:, d, :nb_h].rearrange("p b -> p b ()") \
                    .broadcast_to([128, nb_h, K])

            lnrb = l_nr[:, :nb_h].rearrange("p b -> p b ()").broadcast_to([128, nb_h, K])
            cmb = [cm[:, d, :nb_h].rearrange("p b -> p b ()").broadcast_to([128, nb_h, K])
                   for d in range(3)]

            nc.vector.tensor_copy(ch(13), lnrb)
            ncni = featp.tile([128, 3, half_nb * K], F32, tag="ncni")
            def ncv(d):
                return ncni[:, d, :hw].rearrange("p (b k) -> p b k", k=K)
            for d, g_ in enumerate([gx, gy, gz]):
                nc.vector.tensor_tensor(ncv(d), g_, cmb[d], OP.subtract)
            nc.vector.tensor_tensor(d2, ncv(0), ncv(0), OP.mult)
            nc.vector.tensor_tensor(a0, ncv(1), ncv(1), OP.mult)
            nc.vector.tensor_tensor(d2, d2, a0, OP.add)
            nc.vector.tensor_tensor(a0, ncv(2), ncv(2), OP.mult)
            nc.vector.tensor_tensor(d2, d2, a0, OP.add)
            nc.scalar.activation(ch(14), d2, AF.Sqrt)
            nc.vector.tensor_copy(ch(15), ch(12))

            rr = featp.tile([128, half_nb * K], F32, tag="rr")
            rrv = rr[:, :hw].rearrange("p (b k) -> p b k", k=K)
            nc.vector.tensor_tensor(rrv, lnrb, ch(12), OP.mult)
            emit_angle(ch(16), (nrb(0), nrb(1), nrb(2)), (ch(6), ch(7), ch(8)), rrv)

            nneg = featp.tile([128, 3, half_nb * K], F32, tag="nneg")
            def ngv(d):
                return nneg[:, d, :hw].rearrange("p (b k) -> p b k", k=K)
            nc.vector.tensor_tensor(rrv, ch(14), lnrb, OP.mult)
            for d in range(3):
                nc.vector.tensor_scalar(ngv(d), nrb(d), -1.0, None, OP.mult)
            emit_angle(ch(17), (ncv(0), ncv(1), ncv(2)), (ngv(0), ngv(1), ngv(2)), rrv)

            nc.vector.tensor_tensor(rrv, ch(12), ch(14), OP.mult)
            for d in range(3):
                nc.vector.tensor_scalar(ngv(d), ch(6 + d), -1.0, None, OP.mult)
            nncni = featp.tile([128, 3, half_nb * K], F32, tag="nncni")
            def nnv(d):
                return nncni[:, d, :hw].rearrange("p (b k) -> p b k", k=K)
            for d in range(3):
                nc.vector.tensor_scalar(nnv(d), ncv(d), -1.0, None, OP.mult)
            emit_angle(ch(18), (ngv(0), ngv(1), ngv(2)), (nnv(0), nnv(1), nnv(2)), rrv)



            for bl in range(b0, b1):
                for g in range(NKK_G):
                    tp = pstp.tile([128, 128], F32, tag="tps")
                    inap = F[:, bl - b0, g * 4:(g + 1) * 4, :] \
                        .rearrange("p k c -> p (k c)")
                    nc.tensor.transpose(tp[:], inap, ld["idn"][:])
                    nc.vector.tensor_copy(fcm[:, g, bl * 128:(bl + 1) * 128], tp[:])

    if "fcm" in dbg:
        nc.gpsimd.dma_start(dbg["fcm"][:], fcm[:])

    if "M" not in phases:
        nc.gpsimd.dma_start(out_ap[:, :128], fcm[:96, 0, :128])
        return
    # ================= Phase M =================
    CW = min(512, NI)
    NCH = NI // CW
    with ExitStack() as mctx:
        mlpp = mctx.enter_context(tc.tile_pool(name="mlp", bufs=3))
        stp = mctx.enter_context(tc.tile_pool(name="stats", bufs=1))
        psmm = mctx.enter_context(tc.tile_pool(name="psmm", bufs=2, space="PSUM"))
        psst = mctx.enter_context(tc.tile_pool(name="psst", bufs=1, space="PSUM"))

        scale1 = stp.tile([128, 1], F32); bias1 = stp.tile([128, 1], F32)
        scale2 = stp.tile([128, 1], F32); bias2 = stp.tile([128, 1], F32)
        scale3 = stp.tile([96, 1], F32); bias3 = stp.tile([96, 1], F32)
        bn1 = stp.tile([128, NCH * NKK_G * 2, 6], F32)
        bn2 = stp.tile([128, NCH * NKK_G * 2, 6], F32)
        bn3 = stp.tile([96, NCH * NKK_G * 2 * 2, 6], F32)

        def w1_unit(chunk, g, half, psout):
            lhs = ld["w1a"] if half == 0 else ld["w1b"]
            nc.tensor.matmul(psout[:], lhs[:], fcm[:, g, chunk * CW:(chunk + 1) * CW],
                             start=True, stop=True)

        def norm_relu(ps, sbout, scale, bias, p=128):
            nc.scalar.activation(sbout[:], ps[:], AF.Relu,
                                 bias=bias[:p, :], scale=scale[:p, :])

        def finalize_stats(bn, nunits, nelem_group, m1, expand, grep, brep,
                           scale, bias, parts):
            agg = stp.tile([parts, 2], F32, tag=f"agg{parts}")
            nc.vector.bn_aggr(agg[:], bn[:parts, :nunits, :])
            npe = float(nunits * CW)
            s2 = stp.tile([parts, 2], F32, tag=f"s2{parts}")
            nc.vector.tensor_tensor(s2[:, 1:2], agg[:, 0:1], agg[:, 0:1], OP.mult)
            nc.vector.tensor_tensor(s2[:, 1:2], s2[:, 1:2], agg[:, 1:2], OP.add)
            nc.vector.tensor_copy(s2[:, 0:1], agg[:, 0:1])
            nc.vector.tensor_scalar(s2[:], s2[:], npe, None, OP.mult)
            gps = psst.tile([16, 2], F32, tag="gps")
            nc.tensor.matmul(gps[:], m1[:parts, :], s2[:], start=True, stop=True)
            gsc = stp.tile([16, 2], F32, tag="gsc")
            nc.vector.tensor_copy(gsc[:], gps[:])
            inv_n = 1.0 / float(nelem_group)
            mg = stp.tile([16, 1], F32, tag="mg")
            vg = stp.tile([16, 1], F32, tag="vg")
            t2 = stp.tile([16, 1], F32, tag="t2")
            nc.vector.tensor_scalar(mg[:], gsc[:, 0:1], inv_n, None, OP.mult)
            nc.vector.tensor_scalar(vg[:], gsc[:, 1:2], inv_n, None, OP.mult)
            nc.vector.tensor_tensor(t2[:], mg[:], mg[:], OP.mult)
            nc.vector.tensor_tensor(vg[:], vg[:], t2[:], OP.subtract)
            nc.vector.tensor_scalar(vg[:], vg[:], GN_EPS, None, OP.add)
            nc.vector.reciprocal(vg[:], vg[:])
            nc.scalar.activation(vg[:], vg[:], AF.Sqrt)
            rm = stp.tile([16, 2], F32, tag="rm")
            nc.vector.tensor_copy(rm[:, 0:1], vg[:])
            nc.vector.tensor_copy(rm[:, 1:2], mg[:])
            eps_ = psst.tile([parts, 2], F32, tag="eps")
            nc.tensor.matmul(eps_[:], expand[:, :parts], rm[:], start=True, stop=True)
            rexp = stp.tile([parts, 2], F32, tag=f"rexp{parts}")
            nc.vector.tensor_copy(rexp[:], eps_[:])
            nc.vector.tensor_tensor(scale[:parts, :], rexp[:, 0:1], grep[:parts, :], OP.mult)
            nc.vector.tensor_tensor(bias[:parts, :], rexp[:, 1:2], scale[:parts, :], OP.mult)
            nc.vector.tensor_tensor(bias[:parts, :], brep[:parts, :], bias[:parts, :], OP.subtract)

        # pass A
        for chunk in range(NCH):
            for g in range(NKK_G):
                for half in range(2):
                    ps = psmm.tile([128, CW], F32, tag="psA")
                    w1_unit(chunk, g, half, ps)
                    u = (chunk * NKK_G + g) * 2 + half
                    nc.vector.bn_stats(bn1[:, u, :], ps[:])
        finalize_stats(bn1, NCH * NKK_G * 2, 4 * NIK, ld["m1_12"], ld["e_12"],
                       ld["g1rep"], ld["b1rep"], scale1, bias1, 128)

        # pass B
        for chunk in range(NCH):
            for g in range(NKK_G):
                for half in range(2):
                    ps = psmm.tile([128, CW], F32, tag="psA")
                    w1_unit(chunk, g, half, ps)
                    post = mlpp.tile([128, CW], F32, tag="l1post")
                    norm_relu(ps, post, scale1, bias1)
                    ps2 = psmm.tile([128, CW], F32, tag="psB")
                    nc.tensor.matmul(ps2[:], ld["w2bd"][:], post[:], start=True, stop=True)
                    u = (chunk * NKK_G + g) * 2 + half
                    nc.vector.bn_stats(bn2[:, u, :], ps2[:])
        finalize_stats(bn2, NCH * NKK_G * 2, 4 * NIK, ld["m1_12"], ld["e_12"],
                       ld["g2rep"], ld["b2rep"], scale2, bias2, 128)

        # pass C
        for chunk in range(NCH):
            for g in range(NKK_G):
                for half in range(2):
                    ps = psmm.tile([128, CW], F32, tag="psA")
                    w1_unit(chunk, g, half, ps)
                    post = mlpp.tile([128, CW], F32, tag="l1post")
                    norm_relu(ps, post, scale1, bias1)
                    ps2 = psmm.tile([128, CW], F32, tag="psB")
                    nc.tensor.matmul(ps2[:], ld["w2bd"][:], post[:], start=True, stop=True)
                    post2 = mlpp.tile([128, CW], F32, tag="l2post")
                    norm_relu(ps2, post2, scale2, bias2)
                    for kx in range(2):
                        ps3 = psmm.tile([96, CW], F32, tag="psC")
                        nc.tensor.matmul(ps3[:], ld["w3t"][kx * 64:(kx + 1) * 64, :],
                                         post2[kx * 64:(kx + 1) * 64, :],
                                         start=True, stop=True)
                        u = ((chunk * NKK_G + g) * 2 + half) * 2 + kx
                        nc.vector.bn_stats(bn3[:, u, :], ps3[:])
        finalize_stats(bn3, NCH * NKK_G * 2 * 2, 6 * NIK, ld["m1_3"], ld["e_3"],
                       ld["g3rep"], ld["b3rep"], scale3, bias3, 96)

        # pass D
        outacc = fcmp.tile([96, NI], F32)
        nc.vector.memset(outacc[:], 0.0)
        for chunk in range(NCH):
            for g in range(NKK_G):
                for half in range(2):
                    ps = psmm.tile([128, CW], F32, tag="psA")
                    w1_unit(chunk, g, half, ps)
                    post = mlpp.tile([128, CW], F32, tag="l1post")
                    norm_relu(ps, post, scale1, bias1)
                    ps2 = psmm.tile([128, CW], F32, tag="psB")
                    nc.tensor.matmul(ps2[:], ld["w2bd"][:], post[:], start=True, stop=True)
                    post2 = mlpp.tile([128, CW], F32, tag="l2post")
                    norm_relu(ps2, post2, scale2, bias2)
                    for kx in range(2):
                        ps3 = psmm.tile([96, CW], F32, tag="psC")
                        nc.tensor.matmul(ps3[:], ld["w3t"][kx * 64:(kx + 1) * 64, :],
                                         post2[kx * 64:(kx + 1) * 64, :],
                                         start=True, stop=True)
                        post3 = mlpp.tile([96, CW], F32, tag="l3post")
                        norm_relu(ps3, post3, scale3, bias3, p=96)
                        nc.vector.tensor_tensor(outacc[:, chunk * CW:(chunk + 1) * CW],
                                                outacc[:, chunk * CW:(chunk + 1) * CW],
                                                post3[:], OP.max)

        nc.gpsimd.dma_start(out_ap[:, :NI], outacc[:])


# ======================= SPMD wrapper =======================
import concourse.bacc as bacc
from concourse.bass_utils import run_bass_kernel_spmd

_CACHE = {}


def _build_program():
    if "nc" in _CACHE:
        return _CACHE["nc"]
    nc = bacc.Bacc("TRN2", target_bir_lowering=False, debug=False, num_devices=8)
    din = declare_inputs(nc)
    out_ap = nc.dram_tensor("out", [96, N], F32, kind="ExternalOutput").ap()
    with tile.TileContext(nc) as tc:
        with ExitStack() as ctx:
            build(nc, tc, ctx, din, out_ap)
    nc.compile()
    _CACHE["nc"] = nc
    return nc


def kernel(**inputs):
    data = np.asarray(inputs["data"], dtype=np.float32)
    kk = int(np.asarray(inputs["k"]))
    assert kk == 20 and data.shape == (4, 6, 4096), (data.shape, kk)
    Wn = ["W1", "g1", "b1", "W2", "g2", "b2", "W3", "g3", "b3"]
    Wv = [np.asarray(inputs[n], dtype=np.float32) for n in Wn]
    nc = _build_program()
    in_maps = []
    for core in range(8):
        b = core % 4
        in_maps.append(host_prep(data[b], *Wv))
    res = run_bass_kernel_spmd(nc, in_maps, list(range(8)))
    out = np.stack([res.results[b]["out"] for b in range(4)], axis=0)
    return np.ascontiguousarray(out.astype(np.float32))



# revision 3
# speedup vs baseline: 207.4547x; 207.4547x over previous
"""DGCNN Trainium kernel: per-core one batch (B=4 over 4+4 cores).

Phases per core/batch (sequential, pools scoped per phase):
  S: pd scores via PE (K=4 fp32) -> PSUM (4-bank halves); segmax16 (DVE);
     top-24 segments (max8 rounds); indirect-gather candidate segments;
     DVE rescore; top-20 + global neighbor ids; neighbor gather.
  F: n-major feature math (19 ch) in half batches + PE transpose to
     channel-major F_cm [(kk4,c32) x n].
  M: recompute MLP passes A-D, GN stats via bn_stats + PE-onehot
     reductions, normalize+relu on ACT, final running max over k.
"""
import numpy as np
from contextlib import ExitStack

import concourse.bass as bass
import concourse.tile as tile
from concourse import mybir

dt = mybir.dt
F32, U32, I32 = dt.float32, dt.uint32, dt.int32
AF = mybir.ActivationFunctionType
OP = mybir.AluOpType
AX = mybir.AxisListType

N = 4096
NBLK_FULL = 32
K = 20
SEGW = 16
NSEG = 256
NCS = 24
C1, C2, C3 = 64, 64, 96
GN_EPS = 1e-5
NKK_G = 5
NCAND = NCS * SEGW


def host_prep(data_b, W1, g1, b1, W2, g2, b2, W3, g3, b3):
    """Per-batch host tables (layout prep only). data_b: (6, N) f32."""
    x = data_b[:3].astype(np.float32)
    nrm = data_b[3:6].astype(np.float32)
    xx = ((x[0] * x[0] + x[1] * x[1]) + x[2] * x[2]).astype(np.float32)
    qtab = np.stack([2 * x[0], 2 * x[1], 2 * x[2],
                     np.ones(N, np.float32)]).astype(np.float32)
    rtab = np.stack([x[0], x[1], x[2], -xx]).astype(np.float32)
    seg = np.zeros((NSEG, SEGW, 4), np.float32)
    seg[:, :, 0] = x[0].reshape(NSEG, SEGW)
    seg[:, :, 1] = x[1].reshape(NSEG, SEGW)
    seg[:, :, 2] = x[2].reshape(NSEG, SEGW)
    seg[:, :, 3] = xx.reshape(NSEG, SEGW)
    seg_tab = seg.reshape(NSEG, 64)
    pt = np.zeros((N, 8), np.float32)
    pt[:, 0:3] = x.T
    pt[:, 3:6] = nrm.T
    pt[:, 6] = xx
    ctr = x.T.reshape(NBLK_FULL, 128, 3).transpose(1, 0, 2)
    ctr2 = np.ascontiguousarray(2.0 * ctr).astype(np.float32)
    cnrm = np.ascontiguousarray(nrm.T.reshape(NBLK_FULL, 128, 3).transpose(1, 0, 2))
    iota24 = np.broadcast_to(np.arange(NCS, dtype=np.float32), (128, NCS)).copy()
    idn = np.eye(128, dtype=np.float32)
    W1p = np.zeros((32, C1), np.float32)
    W1p[:19, :] = W1.T
    w1a = np.zeros((128, 128), np.float32)
    w1b = np.zeros((128, 128), np.float32)
    for kk in range(2):
        w1a[kk * 32:(kk + 1) * 32, kk * 64:(kk + 1) * 64] = W1p
        w1b[(kk + 2) * 32:(kk + 3) * 32, kk * 64:(kk + 1) * 64] = W1p
    w2bd = np.zeros((128, 128), np.float32)
    w2bd[:64, :64] = W2.T
    w2bd[64:, 64:] = W2.T
    w3t = np.ascontiguousarray(np.vstack([W3.T, W3.T]))  # [128, 96]
    m1_12 = np.zeros((128, 16), np.float32)
    for p in range(128):
        m1_12[p, (p % 64) // 4] = 1.0
    e_12 = np.zeros((16, 128), np.float32)
    for p in range(128):
        e_12[(p % 64) // 4, p] = 1.0
    m1_3 = np.zeros((96, 16), np.float32)
    for p in range(96):
        m1_3[p, p // 6] = 1.0
    e_3 = np.zeros((16, 96), np.float32)
    for p in range(96):
        e_3[p // 6, p] = 1.0
    g1rep = np.tile(g1, 2).reshape(128, 1).astype(np.float32)
    b1rep = np.tile(b1, 2).reshape(128, 1).astype(np.float32)
    g2rep = np.tile(g2, 2).reshape(128, 1).astype(np.float32)
    b2rep = np.tile(b2, 2).reshape(128, 1).astype(np.float32)
    g3rep = g3.reshape(96, 1).astype(np.float32)
    b3rep = b3.reshape(96, 1).astype(np.float32)
    return {
        "qtab": qtab, "rtab": rtab, "seg_tab": seg_tab, "pt_tab": pt,
        "ctr2": ctr2, "cnrm": cnrm, "iota24": iota24, "idn": idn,
        "w1a": w1a, "w1b": w1b, "w2bd": w2bd, "w3t": w3t,
        "m1_12": m1_12, "e_12": e_12, "m1_3": m1_3, "e_3": e_3,
        "g1rep": g1rep, "b1rep": b1rep, "g2rep": g2rep, "b2rep": b2rep,
        "g3rep": g3rep, "b3rep": b3rep,
    }


INPUT_SHAPES = {
    "qtab": (4, N), "rtab": (4, N), "seg_tab": (NSEG, 64), "pt_tab": (N, 8),
    "ctr2": (128, NBLK_FULL, 3), "cnrm": (128, NBLK_FULL, 3),
    "iota24": (128, NCS), "idn": (128, 128),
    "w1a": (128, 128), "w1b": (128, 128), "w2bd": (128, 128), "w3t": (128, 96),
    "m1_12": (128, 16), "e_12": (16, 128), "m1_3": (96, 16), "e_3": (16, 96),
    "g1rep": (128, 1), "b1rep": (128, 1), "g2rep": (128, 1), "b2rep": (128, 1),
    "g3rep": (96, 1), "b3rep": (96, 1),
}


def declare_inputs(nc):
    return {k: nc.dram_tensor(k, list(sh), F32, kind="ExternalInput").ap()
            for k, sh in INPUT_SHAPES.items()}


def build(nc, tc, ctx, din, out_ap, nblk=NBLK_FULL, dbg=None, phases="SFM"):
    NI = nblk * 128
    NIK = NI * K
    dbg = dbg or {}

    consts = ctx.enter_context(tc.tile_pool(name="consts", bufs=1))
    gathp = ctx.enter_context(tc.tile_pool(name="gath", bufs=1))
    fcmp = ctx.enter_context(tc.tile_pool(name="fcm", bufs=1))

    ld = {}
    for name in ["qtab", "rtab", "ctr2", "cnrm", "iota24", "idn", "w1a", "w1b",
                 "w2bd", "w3t", "m1_12", "e_12", "m1_3", "e_3",
                 "g1rep", "b1rep", "g2rep", "b2rep", "g3rep", "b3rep"]:
        t = consts.tile(list(INPUT_SHAPES[name]), F32, tag=name)
        nc.gpsimd.dma_start(t[:], din[name][:])
        ld[name] = t

    gath = gathp.tile([128, nblk, K, 8], F32)
    fcm = fcmp.tile([128, NKK_G, NI], F32)

    # ================= Phase S =================
    with ExitStack() as sctx:
        selp = sctx.enter_context(tc.tile_pool(name="sel", bufs=2))
        pspd = sctx.enter_context(tc.tile_pool(name="pspd", bufs=1, space="PSUM"))

        import os as _os
        SLVL = int(_os.environ.get("SLVL", "11"))
        for blk in range(nblk):
            segmax = selp.tile([128, NSEG], F32, tag="segmax")
            for h2 in range(2):
                pd = pspd.tile([128, NSEG // 2, SEGW], F32, tag="pd")
                for ch in range(4):
                    nc.tensor.matmul(
                        pd[:].rearrange("p s w -> p (s w)")[:, ch * 512:(ch + 1) * 512],
                        ld["qtab"][:, blk * 128:(blk + 1) * 128],
                        ld["rtab"][:, h2 * 2048 + ch * 512:h2 * 2048 + (ch + 1) * 512],
                        start=True, stop=True)
                nc.vector.tensor_reduce(segmax[:, h2 * 128:(h2 + 1) * 128], pd[:], AX.X, OP.max)

            if SLVL < 2:
                if blk == 0:
                    nc.gpsimd.dma_start(out_ap[:, :NSEG], segmax[:96, :])
                continue
            segv = selp.tile([128, NCS], F32, tag="segv")
            segi = selp.tile([128, NCS], U32, tag="segi")
            for r in range(3):
                nc.vector.max(segv[:, r * 8:(r + 1) * 8], segmax[:])
                nc.vector.max_index(segi[:, r * 8:(r + 1) * 8], segv[:, r * 8:(r + 1) * 8], segmax[:])
                if r < 2:
                    nc.vector.match_replace(segmax[:], segv[:, r * 8:(r + 1) * 8], segmax[:], -1e30)

            segi32 = selp.tile([128, NCS], I32, tag="segi32")
            nc.vector.tensor_copy(segi32[:], segi[:])
            segf = selp.tile([128, NCS], F32, tag="segf")
            nc.vector.tensor_copy(segf[:], segi[:])

            if SLVL < 3:
                if blk == 0:
                    nc.gpsimd.dma_start(out_ap[:, :NCS], segf[:96, :])
                continue
            cand = selp.tile([128, NCS, 64], F32, tag="cand")
            for c in range(NCS):
                nc.gpsimd.indirect_dma_start(
                    out=cand[:, c, :], out_offset=None, in_=din["seg_tab"][:],
                    in_offset=bass.IndirectOffsetOnAxis(ap=segi32[:, c:c + 1], axis=0))

            if SLVL < 4:
                if blk == 0:
                    nc.gpsimd.dma_start(out_ap[:, :64], cand[:96, 0, :])
                continue
            cxyz = cand[:].rearrange("p c (m d) -> p (c m) d", d=4)
            sc = selp.tile([128, NCAND], F32, tag="scores")
            q0 = ld["ctr2"][:, blk, 0:1]
            q1 = ld["ctr2"][:, blk, 1:2]
            q2 = ld["ctr2"][:, blk, 2:3]
            nc.vector.tensor_scalar(sc[:], cxyz[:, :, 0], q0, None, OP.mult)
            nc.vector.scalar_tensor_tensor(sc[:], cxyz[:, :, 1], q1, sc[:], OP.mult, OP.add)
            nc.vector.scalar_tensor_tensor(sc[:], cxyz[:, :, 2], q2, sc[:], OP.mult, OP.add)
            nc.vector.scalar_tensor_tensor(sc[:], cxyz[:, :, 3], -1.0, sc[:], OP.mult, OP.add)

            if SLVL < 5:
                if blk == 0:
                    nc.gpsimd.dma_start(out_ap[:, :NCAND], sc[:96, :])
                continue
            canv = selp.tile([128, NCS], F32, tag="canv")
            cani = selp.tile([128, NCS], U32, tag="cani")
            for r in range(3):
                nc.vector.max(canv[:, r * 8:(r + 1) * 8], sc[:])
                nc.vector.max_index(cani[:, r * 8:(r + 1) * 8], canv[:, r * 8:(r + 1) * 8], sc[:])
                if r < 2:
                    nc.vector.match_replace(sc[:], canv[:, r * 8:(r + 1) * 8], sc[:], -1e30)

            if SLVL < 6:
                if blk == 0:
                    nc.gpsimd.dma_start(out_ap[:, :NCS], canv[:96, :])
                continue
            slot = selp.tile([128, NCS], U32, tag="slot")
            memb = selp.tile([128, NCS], U32, tag="memb")
            nc.vector.tensor_scalar(slot[:], cani[:], 4, None, OP.logical_shift_right)
            nc.vector.tensor_scalar(memb[:], cani[:], 15, None, OP.bitwise_and)
            slotf = selp.tile([128, NCS], F32, tag="slotf")
            membf = selp.tile([128, NCS], F32, tag="membf")
            nc.vector.tensor_copy(slotf[:], slot[:])
            nc.vector.tensor_copy(membf[:], memb[:])
            if SLVL < 7:
                if blk == 0:
                    nc.gpsimd.dma_start(out_ap[:, :NCS], membf[:96, :])
                continue
            eq = selp.tile([128, K, NCS], F32, tag="eq")
            nc.vector.tensor_tensor(
                eq[:], slotf[:, :K].rearrange("p r -> p r ()").broadcast_to([128, K, NCS]),
                ld["iota24"][:].rearrange("p c -> p () c").broadcast_to([128, K, NCS]),
                OP.is_equal)
            nc.vector.tensor_tensor(
                eq[:], eq[:],
                segf[:].rearrange("p c -> p () c").broadcast_to([128, K, NCS]), OP.mult)
            segsel = selp.tile([128, K], F32, tag="segsel")
            nc.vector.tensor_reduce(segsel[:], eq[:], AX.X, OP.add)
            if SLVL < 8:
                if blk == 0:
                    nc.gpsimd.dma_start(out_ap[:, :K], segsel[:96, :])
                continue
            nbrf = selp.tile([128, K], F32, tag="nbrf")
            nc.vector.scalar_tensor_tensor(nbrf[:], segsel[:], 16.0, membf[:, :K], OP.mult, OP.add)
            nbri = selp.tile([128, K], I32, tag="nbri")
            nc.vector.tensor_copy(nbri[:], nbrf[:])

            if SLVL < 9:
                if blk == 0:
                    nc.gpsimd.dma_start(out_ap[:, :K], nbrf[:96, :])
                continue
            if SLVL < 10:
                if blk == 0:
                    nc.gpsimd.dma_start(dbg["nbrf"][:], nbrf[:])
                continue
            if "nbrf" in dbg and blk == 0:
                nc.gpsimd.dma_start(dbg["nbrf"][:], nbrf[:])

            for kk in range(K):
                nc.gpsimd.indirect_dma_start(
                    out=gath[:, blk, kk, :], out_offset=None, in_=din["pt_tab"][:],
                    in_offset=bass.IndirectOffsetOnAxis(ap=nbri[:, kk:kk + 1], axis=0))

    if "gath" in dbg:
        nc.gpsimd.dma_start(dbg["gath"][:], gath[:])

    if "F" not in phases:
        import os as _os2
        if int(_os2.environ.get("SLVL", "11")) >= 11:
            nc.gpsimd.dma_start(out_ap[:, :40], gath[:96, 0, :5, :])
        return
    # ================= Phase F + transposes =================
    half_nb = min(nblk, 8)
    with ExitStack() as fctx:
        featp = fctx.enter_context(tc.tile_pool(name="feat", bufs=1))
        pstp = fctx.enter_context(tc.tile_pool(name="pstp", bufs=2, space="PSUM"))

        for half in range((nblk + half_nb - 1) // half_nb):
            b0 = half * half_nb
            b1 = min(nblk, b0 + half_nb)
            nb_h = b1 - b0
            hw = nb_h * K
            F = featp.tile([128, half_nb, K, 32], F32, tag="F")
            nc.vector.memset(F[:, :, :, 19:32], 0.0)
            t = featp.tile([128, 8, half_nb * K], F32, tag="scratch")

            g3v = gath[:, b0:b1]                       # [p, b, k, 8]
            gx = g3v[:, :, :, 0]; gy = g3v[:, :, :, 1]; gz = g3v[:, :, :, 2]
            nx = g3v[:, :, :, 3]; ny = g3v[:, :, :, 4]; nz = g3v[:, :, :, 5]

            def ctrb(d):
                return ld["ctr2"][:, b0:b1, d:d + 1].broadcast_to([128, nb_h, K])

            def cnb(d):
                return ld["cnrm"][:, b0:b1, d:d + 1].broadcast_to([128, nb_h, K])

            def ch(i):
                return F[:, :nb_h, :, i]

            def tv(i):
                return t[:, i, :hw].rearrange("p (b k) -> p b k", k=K)

            # ch0-2 gxyz ; ch3-5 xc = 0.5*ctr2 ; ch6-8 lxyz
            for d, g_ in enumerate([gx, gy, gz]):
                nc.vector.tensor_copy(ch(d), g_)
                nc.vector.tensor_scalar(ch(3 + d), ctrb(d), 0.5, None, OP.mult)
                nc.vector.tensor_tensor(ch(6 + d), g_, ch(3 + d), OP.subtract)

            def emit_angle(out_ap, v1, v2, r_ap):
                cx_, cy_, cz_, dot_, y2_ = tv(1), tv(2), tv(3), tv(4), tv(5)
                a_, b_ = tv(6), tv(7)
                nc.vector.tensor_tensor(a_, v1[1], v2[2], OP.mult)
                nc.vector.tensor_tensor(b_, v1[2], v2[1], OP.mult)
                nc.vector.tensor_tensor(cx_, a_, b_, OP.subtract)
                nc.vector.tensor_tensor(a_, v1[2], v2[0], OP.mult)
                nc.vector.tensor_tensor(b_, v1[0], v2[2], OP.mult)
                nc.vector.tensor_tensor(cy_, a_, b_, OP.subtract)
                nc.vector.tensor_tensor(a_, v1[0], v2[1], OP.mult)
                nc.vector.tensor_tensor(b_, v1[1], v2[0], OP.mult)
                nc.vector.tensor_tensor(cz_, a_, b_, OP.subtract)
                nc.vector.tensor_tensor(y2_, cx_, cx_, OP.mult)
                nc.vector.tensor_tensor(a_, cy_, cy_, OP.mult)
                nc.vector.tensor_tensor(y2_, y2_, a_, OP.add)
                nc.vector.tensor_tensor(a_, cz_, cz_, OP.mult)
                nc.vector.tensor_tensor(y2_, y2_, a_, OP.add)
                nc.scalar.activation(y2_, y2_, AF.Sqrt)
                nc.vector.tensor_tensor(dot_, v1[0], v2[0], OP.mult)
                nc.vector.tensor_tensor(a_, v1[1], v2[1], OP.mult)
                nc.vector.tensor_tensor(dot_, dot_, a_, OP.add)
                nc.vector.tensor_tensor(a_, v1[2], v2[2], OP.mult)
                nc.vector.tensor_tensor(dot_, dot_, a_, OP.add)
                nc.vector.tensor_tensor(dot_, dot_, r_ap, OP.add)
                nc.vector.tensor_scalar(dot_, dot_, 1e-30, None, OP.max)
                nc.vector.reciprocal(dot_, dot_)
                nc.vector.tensor_tensor(a_, y2_, dot_, OP.mult)
                # q = a_ >= 0 ; atan(q) via two-range identity (ACT domain |x|<=pi/2)
                nc.vector.reciprocal(b_, a_)
                nc.vector.tensor_tensor(cx_, a_, b_, OP.min)
                nc.scalar.activation(out_ap, cx_, AF.Arctan)
                nc.vector.tensor_scalar(b_, a_, 1.0, None, OP.is_gt)
                nc.vector.tensor_scalar(cx_, out_ap, -2.0, 1.5707963267948966, OP.mult, OP.add)
                nc.vector.tensor_tensor(cx_, cx_, b_, OP.mult)
                nc.vector.tensor_tensor(out_ap, out_ap, cx_, OP.add)
                nc.vector.tensor_scalar(out_ap, out_ap, 2.0, None, OP.mult)

            # d_norm (ch12)
            d2 = tv(0)
            a0 = tv(6)
            nc.vector.tensor_tensor(d2, ch(6), ch(6), OP.mult)
            nc.vector.tensor_tensor(a0, ch(7), ch(7), OP.mult)
            nc.vector.tensor_tensor(d2, d2, a0, OP.add)
            nc.vector.tensor_tensor(a0, ch(8), ch(8), OP.mult)
            nc.vector.tensor_tensor(d2, d2, a0, OP.add)
            nc.scalar.activation(ch(12), d2, AF.Sqrt)

            emit_angle(ch(9), (cnb(0), cnb(1), cnb(2)), (ch(6), ch(7), ch(8)), ch(12))
            emit_angle(ch(10), (nx, ny, nz), (ch(6), ch(7), ch(8)), ch(12))
            ones_ = featp.tile([128, 1], F32, tag="ones")
            nc.vector.memset(ones_[:], 1.0)
            emit_angle(ch(11), (cnb(0), cnb(1), cnb(2)), (nx, ny, nz),
                       ones_[:].rearrange("p o -> p o ()").broadcast_to([128, nb_h, K]))

            cm = featp.tile([128, 3, half_nb], F32, tag="cm")
            nr = featp.tile([128, 3, half_nb], F32, tag="nr")
            l_nr = featp.tile([128, half_nb], F32, tag="lnr")
            l2 = featp.tile([128, half_nb], F32, tag="l2")
            for d, g_ in enumerate([gx, gy, gz]):
                nc.vector.tensor_reduce(cm[:, d, :nb_h], g_, AX.X, OP.add)
                nc.vector.tensor_scalar(cm[:, d, :nb_h], cm[:, d, :nb_h], 1.0 / K, None, OP.mult)
                nc.vector.scalar_tensor_tensor(nr[:, d, :nb_h], ld["ctr2"][:, b0:b1, d], -0.5,
                                               cm[:, d, :nb_h], OP.mult, OP.add)
            nc.vector.tensor_tensor(l2[:, :nb_h], nr[:, 0, :nb_h], nr[:, 0, :nb_h], OP.mult)
            nc.vector.tensor_tensor(l_nr[:, :nb_h], nr[:, 1, :nb_h], nr[:, 1, :nb_h], OP.mult)
            nc.vector.tensor_tensor(l2[:, :nb_h], l2[:, :nb_h], l_nr[:, :nb_h], OP.add)
            nc.vector.tensor_tensor(l_nr[:, :nb_h], nr[:, 2, :nb_h], nr[:, 2, :nb_h], OP.mult)
            nc.vector.tensor_tensor(l2[:, :nb_h], l2[:, :nb_h], l_nr[:, :nb_h], OP.add)
            nc.scalar.activation(l_nr[:, :nb_h], l2[:, :nb_h], AF.Sqrt)

            def nrb(d):
                return nr[:, d, :nb_h].rearrange("p b -> p b ()") \
                    .broadcast_to([128, nb_h, K])

            lnrb = l_nr[:, :nb_h].rearrange("p b -> p b ()").broadcast_to([128, nb_h, K])
            cmb = [cm[:, d, :nb_h].rearrange("p b -> p b ()").broadcast_to([128, nb_h, K])
                   for d in range(3)]

            nc.vector.tensor_copy(ch(13), lnrb)
            ncni = featp.tile([128, 3, half_nb * K], F32, tag="ncni")
            def ncv(d):
                return ncni[:, d, :hw].rearrange("p (b k) -> p b k", k=K)
            for d, g_ in enumerate([gx, gy, gz]):
                nc.vector.tensor_tensor(ncv(d), g_, cmb[d], OP.subtract)
            nc.vector.tensor_tensor(d2, ncv(0), ncv(0), OP.mult)
            nc.vector.tensor_tensor(a0, ncv(1), ncv(1), OP.mult)
            nc.vector.tensor_tensor(d2, d2, a0, OP.add)
            nc.vector.tensor_tensor(a0, ncv(2), ncv(2), OP.mult)
            nc.vector.tensor_tensor(d2, d2, a0, OP.add)
            nc.scalar.activation(ch(14), d2, AF.Sqrt)
            nc.vector.tensor_copy(ch(15), ch(12))

            rr = featp.tile([128, half_nb * K], F32, tag="rr")
            rrv = rr[:, :hw].rearrange("p (b k) -> p b k", k=K)
            nc.vector.tensor_tensor(rrv, lnrb, ch(12), OP.mult)
            emit_angle(ch(16), (nrb(0), nrb(1), nrb(2)), (ch(6), ch(7), ch(8)), rrv)

            nneg = featp.tile([128, 3, half_nb * K], F32, tag="nneg")
            def ngv(d):
                return nneg[:, d, :hw].rearrange("p (b k) -> p b k", k=K)
            nc.vector.tensor_tensor(rrv, ch(14), lnrb, OP.mult)
            for d in range(3):
                nc.vector.tensor_scalar(ngv(d), nrb(d), -1.0, None, OP.mult)
            emit_angle(ch(17), (ncv(0), ncv(1), ncv(2)), (ngv(0), ngv(1), ngv(2)), rrv)

            nc.vector.tensor_tensor(rrv, ch(12), ch(14), OP.mult)
            for d in range(3):
                nc.vector.tensor_scalar(ngv(d), ch(6 + d), -1.0, None, OP.mult)
            nncni = featp.tile([128, 3, half_nb * K], F32, tag="nncni")
            def nnv(d):
                return nncni[:, d, :hw].rearrange("p (b k) -> p b k", k=K)
            for d in range(3):
                nc.vector.tensor_scalar(nnv(d), ncv(d), -1.0, None, OP.mult)
            emit_angle(ch(18), (ngv(0), ngv(1), ngv(2)), (nnv(0), nnv(1), nnv(2)), rrv)



            for bl in range(b0, b1):
                for g in range(NKK_G):
                    tp = pstp.tile([128, 128], F32, tag="tps")
                    inap = F[:, bl - b0, g * 4:(g + 1) * 4, :] \
                        .rearrange("p k c -> p (k c)")
                    nc.tensor.transpose(tp[:], inap, ld["idn"][:])
                    nc.vector.tensor_copy(fcm[:, g, bl * 128:(bl + 1) * 128], tp[:])

    if "fcm" in dbg:
        nc.gpsimd.dma_start(dbg["fcm"][:], fcm[:])

    if "M" not in phases:
        nc.gpsimd.dma_start(out_ap[:, :128], fcm[:96, 0, :128])
        return
    # ================= Phase M =================
    CW = min(512, NI)
    NCH = NI // CW
    with ExitStack() as mctx:
        mlpp = mctx.enter_context(tc.tile_pool(name="mlp", bufs=3))
        stp = mctx.enter_context(tc.tile_pool(name="stats", bufs=1))
        psmm = mctx.enter_context(tc.tile_pool(name="psmm", bufs=2, space="PSUM"))
        psst = mctx.enter_context(tc.tile_pool(name="psst", bufs=1, space="PSUM"))

        scale1 = stp.tile([128, 1], F32); bias1 = stp.tile([128, 1], F32)
        scale2 = stp.tile([128, 1], F32); bias2 = stp.tile([128, 1], F32)
        scale3 = stp.tile([96, 1], F32); bias3 = stp.tile([96, 1], F32)
        bn1 = stp.tile([128, NCH * NKK_G * 2, 6], F32)
        bn2 = stp.tile([128, NCH * NKK_G * 2, 6], F32)
        bn3 = stp.tile([96, NCH * NKK_G * 2 * 2, 6], F32)

        def w1_unit(chunk, g, half, psout):
            lhs = ld["w1a"] if half == 0 else ld["w1b"]
            nc.tensor.matmul(psout[:], lhs[:], fcm[:, g, chunk * CW:(chunk + 1) * CW],
                             start=True, stop=True)

        def norm_relu(ps, sbout, scale, bias, p=128):
            nc.scalar.activation(sbout[:], ps[:], AF.Relu,
                                 bias=bias[:p, :], scale=scale[:p, :])

        def finalize_stats(bn, nunits, nelem_group, m1, expand, grep, brep,
                           scale, bias, parts):
            agg = stp.tile([parts, 2], F32, tag=f"agg{parts}")
            nc.vector.bn_aggr(agg[:], bn[:parts, :nunits, :])
            npe = float(nunits * CW)
            s2 = stp.tile([parts, 2], F32, tag=f"s2{parts}")
            nc.vector.tensor_tensor(s2[:, 1:2], agg[:, 0:1], agg[:, 0:1], OP.mult)
            nc.vector.tensor_tensor(s2[:, 1:2], s2[:, 1:2], agg[:, 1:2], OP.add)
            nc.vector.tensor_copy(s2[:, 0:1], agg[:, 0:1])
            nc.vector.tensor_scalar(s2[:], s2[:], npe, None, OP.mult)
            gps = psst.tile([16, 2], F32, tag="gps")
            nc.tensor.matmul(gps[:], m1[:parts, :], s2[:], start=True, stop=True)
            gsc = stp.tile([16, 2], F32, tag="gsc")
            nc.vector.tensor_copy(gsc[:], gps[:])
            inv_n = 1.0 / float(nelem_group)
            mg = stp.tile([16, 1], F32, tag="mg")
            vg = stp.tile([16, 1], F32, tag="vg")
            t2 = stp.tile([16, 1], F32, tag="t2")
            nc.vector.tensor_scalar(mg[:], gsc[:, 0:1], inv_n, None, OP.mult)
            nc.vector.tensor_scalar(vg[:], gsc[:, 1:2], inv_n, None, OP.mult)
            nc.vector.tensor_tensor(t2[:], mg[:], mg[:], OP.mult)
            nc.vector.tensor_tensor(vg[:], vg[:], t2[:], OP.subtract)
            nc.vector.tensor_scalar(vg[:], vg[:], GN_EPS, None, OP.add)
            nc.vector.reciprocal(vg[:], vg[:])
            nc.scalar.activation(vg[:], vg[:], AF.Sqrt)
            rm = stp.tile([16, 2], F32, tag="rm")
            nc.vector.tensor_copy(rm[:, 0:1], vg[:])
            nc.vector.tensor_copy(rm[:, 1:2], mg[:])
            eps_ = psst.tile([parts, 2], F32, tag="eps")
            nc.tensor.matmul(eps_[:], expand[:, :parts], rm[:], start=True, stop=True)
            rexp = stp.tile([parts, 2], F32, tag=f"rexp{parts}")
            nc.vector.tensor_copy(rexp[:], eps_[:])
            nc.vector.tensor_tensor(scale[:parts, :], rexp[:, 0:1], grep[:parts, :], OP.mult)
            nc.vector.tensor_tensor(bias[:parts, :], rexp[:, 1:2], scale[:parts, :], OP.mult)
            nc.vector.tensor_tensor(bias[:parts, :], brep[:parts, :], bias[:parts, :], OP.subtract)

        # pass A
        for chunk in range(NCH):
            for g in range(NKK_G):
                for half in range(2):
                    ps = psmm.tile([128, CW], F32, tag="psA")
                    w1_unit(chunk, g, half, ps)
                    u = (chunk * NKK_G + g) * 2 + half
                    nc.vector.bn_stats(bn1[:, u, :], ps[:])
        finalize_stats(bn1, NCH * NKK_G * 2, 4 * NIK, ld["m1_12"], ld["e_12"],
                       ld["g1rep"], ld["b1rep"], scale1, bias1, 128)

        # pass B
        for chunk in range(NCH):
            for g in range(NKK_G):
                for half in range(2):
                    ps = psmm.tile([128, CW], F32, tag="psA")
                    w1_unit(chunk, g, half, ps)
                    post = mlpp.tile([128, CW], F32, tag="l1post")
                    norm_relu(ps, post, scale1, bias1)
                    ps2 = psmm.tile([128, CW], F32, tag="psB")
                    nc.tensor.matmul(ps2[:], ld["w2bd"][:], post[:], start=True, stop=True)
                    u = (chunk * NKK_G + g) * 2 + half
                    nc.vector.bn_stats(bn2[:, u, :], ps2[:])
        finalize_stats(bn2, NCH * NKK_G * 2, 4 * NIK, ld["m1_12"], ld["e_12"],
                       ld["g2rep"], ld["b2rep"], scale2, bias2, 128)

        # pass C
        for chunk in range(NCH):
            for g in range(NKK_G):
                for half in range(2):
                    ps = psmm.tile([128, CW], F32, tag="psA")
                    w1_unit(chunk, g, half, ps)
                    post = mlpp.tile([128, CW], F32, tag="l1post")
                    norm_relu(ps, post, scale1, bias1)
                    ps2 = psmm.tile([128, CW], F32, tag="psB")
                    nc.tensor.matmul(ps2[:], ld["w2bd"][:], post[:], start=True, stop=True)
                    post2 = mlpp.tile([128, CW], F32, tag="l2post")
                    norm_relu(ps2, post2, scale2, bias2)
                    for kx in range(2):
                        ps3 = psmm.tile([96, CW], F32, tag="psC")
                        nc.tensor.matmul(ps3[:], ld["w3t"][kx * 64:(kx + 1) * 64, :],
                                         post2[kx * 64:(kx + 1) * 64, :],
                                         start=True, stop=True)
                        u = ((chunk * NKK_G + g) * 2 + half) * 2 + kx
                        nc.vector.bn_stats(bn3[:, u, :], ps3[:])
        finalize_stats(bn3, NCH * NKK_G * 2 * 2, 6 * NIK, ld["m1_3"], ld["e_3"],
                       ld["g3rep"], ld["b3rep"], scale3, bias3, 96)

        # pass D
        outacc = fcmp.tile([96, NI], F32)
        nc.vector.memset(outacc[:], 0.0)
        for chunk in range(NCH):
            for g in range(NKK_G):
                for half in range(2):
                    ps = psmm.tile([128, CW], F32, tag="psA")
                    w1_unit(chunk, g, half, ps)
                    post = mlpp.tile([128, CW], F32, tag="l1post")
                    norm_relu(ps, post, scale1, bias1)
                    ps2 = psmm.tile([128, CW], F32, tag="psB")
                    nc.tensor.matmul(ps2[:], ld["w2bd"][:], post[:], start=True, stop=True)
                    post2 = mlpp.tile([128, CW], F32, tag="l2post")
                    norm_relu(ps2, post2, scale2, bias2)
                    for kx in range(2):
                        ps3 = psmm.tile([96, CW], F32, tag="psC")
                        nc.tensor.matmul(ps3[:], ld["w3t"][kx * 64:(kx + 1) * 64, :],
                                         post2[kx * 64:(kx + 1) * 64, :],
                                         start=True, stop=True)
                        post3 = mlpp.tile([96, CW], F32, tag="l3post")
                        norm_relu(ps3, post3, scale3, bias3, p=96)
                        nc.vector.tensor_tensor(outacc[:, chunk * CW:(chunk + 1) * CW],
                                                outacc[:, chunk * CW:(chunk + 1) * CW],
                                                post3[:], OP.max)

        nc.gpsimd.dma_start(out_ap[:, :NI], outacc[:])


# ======================= SPMD wrapper =======================
import concourse.bacc as bacc
from concourse.bass_utils import run_bass_kernel_spmd

_CACHE = {}


def _build_program():
    if "nc" in _CACHE:
        return _CACHE["nc"]
    nc = bacc.Bacc("TRN2", target_bir_lowering=False, debug=False, num_devices=8)
    din = declare_inputs(nc)
    out_ap = nc.dram_tensor("out", [96, N], F32, kind="ExternalOutput").ap()
    import os as _os3
    with tile.TileContext(nc) as tc:
        with ExitStack() as ctx:
            build(nc, tc, ctx, din, out_ap,
                  nblk=int(_os3.environ.get("NBLK", str(NBLK_FULL))),
                  phases=_os3.environ.get("PHASES", "SFM"))
    nc.compile()
    _CACHE["nc"] = nc
    return nc


def kernel(**inputs):
    data = np.asarray(inputs["data"], dtype=np.float32)
    kk = int(np.asarray(inputs["k"]))
    assert kk == 20 and data.shape == (4, 6, 4096), (data.shape, kk)
    Wn = ["W1", "g1", "b1", "W2", "g2", "b2", "W3", "g3", "b3"]
    Wv = [np.asarray(inputs[n], dtype=np.float32) for n in Wn]
    nc = _build_program()
    in_maps = []
    for core in range(8):
        b = core % 4
        in_maps.append(host_prep(data[b], *Wv))
    res = run_bass_kernel_spmd(nc, in_maps, list(range(8)))
    out = np.stack([res.results[b]["out"] for b in range(4)], axis=0)
    return np.ascontiguousarray(out.astype(np.float32))

